# revision 34
# baseline (speedup 1.0000x reference)
"""Causal self-attention (B=4, T=2048, C=1024, H=16) on 8 trn2 NeuronCores.

Sharding: tensor-parallel over heads x data-parallel over batch.
Core c handles batch b=c//2 and head group g=c%2 (8 heads each).
Each core computes qkv projection for its heads, causal attention, and a
partial output projection; the host sums the two partial yT per batch and
adds the output bias.

Device dataflow is feature-major ("transposed") end to end:
  qkT[f, t]   = Wqk.T @ xT          (f = head-pair-blocked q/k features)
  scoresT[k, q] = kT.T @ qT         per head, k-tile=128 x q-tile=512
  e = exp(scoresT/8); diagonal blocks then get their upper triangle zeroed
      on the DVE (bf16 multiply with a keep-mask) before av consumes them
  avT[d(+1), q] += aug(v).T @ e     ones-column gives softmax denominator;
      even head lands at PSUM partitions 0-64 ([v|1]); odd head uses
      [1|0*63|v] (M=128) so its denominator lands at partition 0 and its d
      rows at 64-127 — the whole norm is then per-partition DVE work (no
      partition-hop DMA) and both K=1 denominator broadcasts read from
      legal array-tile base partitions (64 resp. 0)
  aoT = avT_d * (1/denom); the denominators broadcast over their head's
      partitions via K=128 one-hot-row selector matmuls (plain 128-row
      mode; K=1 outer products cannot write PSUM partitions 64-127, and
      the custom-DVE reciprocal no-ops on base-partition-64 outputs, so
      reciprocals run on base-0 ranges)
  yT_partial = Wo.T @ aoT           stored fp16, host sums the two partials
No transposes are needed anywhere; the host transposes x and y (free).
Heads are packed two per 128-partition block (even head at partitions 0-63,
odd at 64-127).

Performance structure (beyond the bf16 baseline):
  - q/k projection runs in fp8 (float8e4) with DoubleRow perf mode: x and
    32*Wqk ship as fp8, each matmul contracts 256 features; the 1/32
    unscale folds into the bias tensor_scalar. Attention stays bf16.
  - The PE's 64-row tiling mode (scores, K=64, auto row-paired E/O heads)
    and 128-row mode (av/qkv/proj) alternate; each mode switch drains the
    array (~100ns). The kt loop runs in 2-kt batches: both kts' score
    matmuls issue back-to-back in 64-row mode, then all 128-row work
    (deferred av, chunk/proj items) follows, halving switch count.
  - Causal masking costs no PE work: exp runs on the raw scores (scores/8
    is O(1), no overflow) and the DVE zeroes the masked upper triangle of
    the diagonal 128-col block in e (bf16, 2x DVE rate) before av reads.
  - Per-k-tile column restriction: for diagonal k-tiles only columns
    [k0-q0:] of the 512-query block are computed in scores/exp/av.
  - Work scheduling balances PE vs the exp-bound scalar engine: chunk
    (next qt's qkv) items spread evenly over the current qt's k-tiles via
    a density credit; all projection work is deferred to the qt3 window
    (qt3 has no chunk work and would otherwise starve the PE while ACT
    grinds exp). y PSUM->SBUF copies run on the DVE (fp16), except the
    post-last-norm epilogue where they alternate DVE/ACT; epilogue y
    stores alternate the sync/gpsimd DMA queues.
  - All DRAM tensors are host-prepacked into device access order with the
    partition dim INSIDE the sliced dims, so every DMA transfer is one
    fully contiguous 128KB-1MB block (naive [C,T] layouts produced 0.5-1KB
    strided descriptors that throttled the queues to ~70GB/s and made
    descriptor generation the startup bottleneck).
  - Startup DMA: x (bf16+fp8) on the sync queue, weights on the gpsimd
    queue, wo after the chunk-1 x loads. Later chunk loads split by need
    order: bf16 halves (v items consume them first) on sync, fp8 halves
    (qk items, popped later) on gpsimd behind the weights.
"""

import os
import threading
from contextlib import ExitStack

import ml_dtypes
import numpy as np

import concourse.bass as bass
from concourse import bacc
import concourse.mybir as mybir
import concourse.tile as tile
from concourse.bass_utils import run_bass_kernel_spmd

B, T, C = 4, 2048, 1024
H, D = 16, 64
NCORES = 8
HL = 8                 # heads per core
NPAIR = HL // 2        # head pairs per core
CQK = 2 * HL * D       # 1024 local q+k features
CV = HL * D            # 512 local v features
TQ = 512               # query tile (PSUM bank limit for f32)
NQT = T // TQ          # 4
TK = 128               # key tile (PSUM partition limit)
NKT = T // TK          # 16
KO = C // 128          # 8 contraction tiles over C
KH = KO // 2           # half-contraction (4 ko) for split loads
F32 = mybir.dt.float32
BF16 = mybir.dt.bfloat16
FP16 = mybir.dt.float16
FP8 = mybir.dt.float8e4
DRM = mybir.MatmulPerfMode.DoubleRow

# float32r: full-precision fp32 data, fast PE streaming mode (1 cycle/row at
# N>=256 vs 4 for plain float32).
MM_DT = {
    "f32r": mybir.dt.float32r,
    "f32": mybir.dt.float32,
}[os.environ.get("ATTN_MM_DT", "f32r")]


def r(ap):
    """View an fp32 AP as the matmul input dtype (float32r needs producers to
    write through an fp32r-typed AP so the BIR verifier sees rounded data)."""
    if MM_DT == F32 or ap.dtype != F32:
        return ap
    return ap.bitcast(MM_DT)


def _mm(nc, out, lhsT, rhs, start=True, stop=True):
    nc.tensor.matmul(out, r(lhsT), r(rhs), start=start, stop=stop)


def build_program():
    nc = bacc.Bacc(None)
    # All inputs are host-prepacked into device access order; the partition
    # dim sits inside the sliced dims so each DMA is one contiguous block.
    xT = nc.declare_dram_parameter("xT", [NQT, 2, 128, KH, TQ], BF16, isOutput=False)
    x8T = nc.declare_dram_parameter("x8T", [NQT, 2, 128, KH, TQ], FP8, isOutput=False)
    wqk = nc.declare_dram_parameter("wqk", [128, KO, CQK], FP8, isOutput=False)
    bqk = nc.declare_dram_parameter("bqk", [128, 8], F32, isOutput=False)
    wv = nc.declare_dram_parameter("wv", [2, 128, KH, CV], BF16, isOutput=False)
    bv = nc.declare_dram_parameter("bv", [CV], F32, isOutput=False)
    wo = nc.declare_dram_parameter("wo", [128, 4, C], BF16, isOutput=False)
    yT = nc.declare_dram_parameter("yT", [NQT, 8, 128, TQ], FP16, isOutput=True)

    with ExitStack() as ctx:
        ctx.enter_context(nc.allow_low_precision(reason="fp32r matmul inputs"))
        tc = ctx.enter_context(tile.TileContext(nc))
        persist = ctx.enter_context(tc.tile_pool(name="persist", bufs=1))
        p2 = ctx.enter_context(tc.tile_pool(name="p2", bufs=3))
        pw = ctx.enter_context(tc.tile_pool(name="pw", bufs=1))
        px = ctx.enter_context(tc.tile_pool(name="px", bufs=2))
        ps = ctx.enter_context(tc.tile_pool(name="ps", bufs=2, space="PSUM"))
        ps_acc = ctx.enter_context(tc.tile_pool(name="ps_acc", bufs=2, space="PSUM"))
        ps_av = ctx.enter_context(tc.tile_pool(name="ps_av", bufs=1, space="PSUM"))

        # q/k features, head-pair blocked: block m<4 = q of pair m
        # (even head partitions 0-63, odd 64-127), block 4+m = k of pair m.
        # One tile per 512-token chunk so chunk writes and attention reads
        # of different chunks never false-serialize (deps are per-tile).
        qkTs = [persist.tile([128, 8, TQ], BF16, name=f"qkT{c}")
                for c in range(NQT)]
        # v with ones column for the softmax denominator:
        # [tok, kt, head, 128]; even heads [v|1|0*63] (av reads cols 0:65),
        # odd heads [1|0*63|v] (av reads all 128 cols) so the av outer
        # product lands the odd head's denominator at PSUM partition 0 and
        # its d rows at partitions 64-127.
        v_augs = [persist.tile([128, TQ // TK, HL, 128], BF16,
                               name=f"vaug{c}") for c in range(NQT)]
        bqk_sb = persist.tile([128, 8], F32)
        bv_row = persist.tile([1, CV], F32)
        bvb_sb = persist.tile([128, CV], F32)    # v bias broadcast over tokens
        ones_sb = persist.tile([128, 128], F32)
        wo_sb = persist.tile([128, 4, C], BF16)
        # normalized attention output, one tile per head pair (per-tile deps:
        # the projection's per-ko reads then only wait on that pair's norm)
        aoTs = [persist.tile([128, T], BF16, name=f"aoT{p}")
                for p in range(NPAIR)]

        # triK[k, h, q] = 1 if q >= k else 0: causal keep-mask for zeroing
        # the diagonal block of e after exp (both halves share the pattern).
        triK = persist.tile([128, 2, 128], BF16)
        ones_f32 = persist.tile([128, 128], F32)
        nc.vector.memset(ones_f32, 1.0)
        nc.vector.tensor_copy(out=r(ones_sb[:]), in_=ones_f32)
        iot = persist.tile([128, 128], F32)
        iop = persist.tile([128, 128], F32)
        nc.gpsimd.iota(iot, pattern=[[1, 128]], base=0, channel_multiplier=0,
                       allow_small_or_imprecise_dtypes=True)
        nc.gpsimd.iota(iop, pattern=[[0, 128]], base=0, channel_multiplier=1,
                       allow_small_or_imprecise_dtypes=True)
        scr = persist.tile([128, 128], F32)
        nc.vector.tensor_tensor(out=scr, in0=iot, in1=iop,
                                op=mybir.AluOpType.is_ge)
        # sel0[j, m] = [j == 0]: K=128 row-selector that broadcasts SBUF
        # partition 0 (the odd-head denominator) to all 128 PSUM partitions.
        # (A K=1 M=128 outer product cannot write PSUM partitions 64-127:
        # row-tiled small-K matmuls never reach the high PSUM half.)
        sel0 = persist.tile([128, 128], BF16)
        nc.vector.tensor_scalar(out=sel0, in0=iop, scalar1=0.0,
                                scalar2=None, op0=mybir.AluOpType.is_equal)
        sel64 = persist.tile([128, 128], BF16)
        nc.vector.tensor_scalar(out=sel64, in0=iop, scalar1=float(D),
                                scalar2=None, op0=mybir.AluOpType.is_equal)
        for h in range(2):
            nc.vector.tensor_copy(out=triK[:, h, :], in_=scr)
        nc.sync.dma_start(out=bqk_sb, in_=bqk[:])
        nc.sync.dma_start(out=r(bv_row[:]), in_=r(bv[:].unsqueeze(0)))

        # chunk-0 x and the v weights load first (ko-halves for finer deps)
        # so the first v matmuls start as early as possible.
        xt0 = [px.tile([128, KH, TQ], BF16, name=f"xt0_{h}", tag=f"xt{h}")
               for h in range(2)]
        xt80 = [px.tile([128, KH, TQ], FP8, name=f"x8t0_{h}", tag=f"x8{h}")
                for h in range(2)]
        wv_sb = [pw.tile([128, KH, CV], BF16, name=f"wv_{h}", tag=f"wv{h}")
                 for h in range(2)]
        # two parallel DMA queues for the startup loads; h=0 halves first so
        # the first (half-contraction) v matmuls start after ~1MB, not 4MB
        wqk_sb = pw.tile([128, KO, CQK], FP8)
        # flatten to [p, kk*tt]: the sync queue's HW descriptor generator
        # does not coalesce across AP dims (1KB packets); a 2-D AP gives it
        # full 4KB/2KB per-partition runs
        def flat(ap):
            return ap.rearrange("p a b -> p (a b)")
        for h in range(2):
            nc.sync.dma_start(out=flat(xt0[h][:]), in_=flat(xT[0, h]))
            nc.gpsimd.dma_start(out=r(wv_sb[h][:]), in_=r(wv[h]))
        for h in range(2):
            nc.sync.dma_start(out=flat(xt80[h][:]), in_=flat(x8T[0, h]))
        for h in range(2):  # halves: the first qk matmuls need only ko 0-3
            nc.gpsimd.dma_start(out=wqk_sb[:, h * KH : (h + 1) * KH, :],
                                in_=wqk[:, h * KH : (h + 1) * KH, :])
        # v_aug ones/pad init on the (otherwise idle) gpsimd engine, emitted
        # after the weight-DMA descriptor generation so it does not delay
        # the startup loads; pads only (v values overwrite the rest).
        for c in range(NQT):
            va = v_augs[c][:].rearrange("p a (h2 e) x -> p a h2 e x", e=2)
            nc.gpsimd.memset(va[:, :, :, 0, D + 1 : 128], 0.0)
            nc.gpsimd.memset(va[:, :, :, 1, 1:D], 0.0)
            nc.gpsimd.tensor_copy(   # even heads: ones in column D
                out=va[:, :, :, 0, D : D + 1],
                in_=ones_f32[:, 0 : (TQ // TK) * NPAIR].rearrange(
                    "p (a b c) -> p a b c", a=TQ // TK, b=NPAIR))
            nc.gpsimd.tensor_copy(   # odd heads: ones in column 0
                out=va[:, :, :, 1, 0:1],
                in_=ones_f32[:, 0 : (TQ // TK) * NPAIR].rearrange(
                    "p (a b c) -> p a b c", a=TQ // TK, b=NPAIR))

        # v-bias broadcast over the 128 token partitions via K=1 outer product.
        # The first 7 are dummies: they only keep the PE busy from ~8us so
        # the HAM clock-gate is warm (2.4GHz) when the x/wv loads land.
        bvb_ps = ps_acc.tile([128, CV], F32, tag="acc")
        for _ in range(7):
            _mm(nc, bvb_ps, ones_sb[0:1, :], bv_row)
        _mm(nc, bvb_ps, ones_sb[0:1, :], bv_row)
        nc.vector.tensor_copy(out=bvb_sb, in_=bvb_ps)

        def qkv_chunk_items(ch, xt, xt8, split_v=False):
            """Per-chunk QKV work, as one closure per matmul group."""

            def v_mt(mt, kos=range(KO), acc_in=None):
                def f():
                    acc = acc_in or ps_acc.tile([128, CV], F32, tag="acc")
                    for ko in kos:
                        _mm(nc, acc,
                            xt[ko // KH][:, ko % KH, mt * TK : (mt + 1) * TK],
                            wv_sb[ko // KH][:, ko % KH, :],
                            start=ko == 0, stop=ko == KO - 1)
                    if kos[-1] == KO - 1:
                        # v + bias, even heads into cols 0:D, odd into 64:128
                        accv = acc.rearrange("p (h2 e d) -> p h2 e d", e=2, d=D)
                        bvv = bvb_sb.rearrange("p (h2 e d) -> p h2 e d", e=2, d=D)
                        vav = v_augs[ch][:, mt].rearrange(
                            "p (h2 e) x -> p h2 e x", e=2)
                        nc.vector.tensor_add(
                            out=vav[:, :, 0, 0:D], in0=accv[:, :, 0, :],
                            in1=bvv[:, :, 0, :])
                        nc.vector.tensor_add(
                            out=vav[:, :, 1, 64:128], in0=accv[:, :, 1, :],
                            in1=bvv[:, :, 1, :])
                    return acc
                return f

            def qk_m(m, irange=(0, 1, 2, 3), acc_in=None):
                def f():
                    acc = acc_in or ps_acc.tile([128, TQ], F32, tag="acc")
                    for i in irange:  # fp8 DoubleRow over ko pairs
                        nc.tensor.matmul(
                            acc, wqk_sb[:, 2 * i : 2 * i + 2,
                                        m * 128 : (m + 1) * 128],
                            xt8[i // 2][:, (2 * i) % KH : (2 * i) % KH + 2, :],
                            start=i == 0, stop=i == 3, perf_mode=DRM)
                    if irange[-1] == 3:
                        # qkT = acc/32 + b (wqk is stored as 32*W in fp8)
                        nc.vector.tensor_scalar(
                            out=qkTs[ch][:, m, :], in0=acc,
                            scalar1=1.0 / 32.0, scalar2=bqk_sb[:, m : m + 1],
                            op0=mybir.AluOpType.mult, op1=mybir.AluOpType.add)
                    return acc
                return f

            if split_v:
                # half-contraction split: the A halves need only the h=0
                # loads, so compute starts while h=1 is still in flight;
                # exposes (a_items, b_items, qk_m) for custom scheduling
                accs = {}
                def mk_a(mt):
                    return lambda: accs.__setitem__(
                        mt, v_mt(mt, kos=list(range(KH)))())
                def mk_b(mt):
                    return lambda: v_mt(mt, kos=list(range(KH, KO)),
                                        acc_in=accs[mt])()
                return ([mk_a(m) for m in range(4)],
                        [mk_b(m) for m in range(4)], qk_m)
            return [v_mt(mt) for mt in range(TQ // TK)] + \
                   [qk_m(m) for m in range(8)]

        def load_chunk(ch):
            xt = [px.tile([128, KH, TQ], BF16, name=f"xt_{ch}_{h}", tag=f"xt{h}")
                  for h in range(2)]
            xt8 = [px.tile([128, KH, TQ], FP8, name=f"x8t_{ch}_{h}",
                           tag=f"x8{h}") for h in range(2)]
            # bf16 halves (v items need them first) on sync; fp8 halves
            # (qk items, popped later) on gpsimd behind the weight loads
            def flat(ap):
                return ap.rearrange("p a b -> p (a b)")
            for h in range(2):
                nc.sync.dma_start(out=flat(xt[h][:]), in_=flat(xT[ch, h]))
            for h in range(2):
                nc.gpsimd.dma_start(out=flat(xt8[h][:]), in_=flat(x8T[ch, h]))
            return xt, xt8

        # chunk 0 prologue: only what qt0 pair 0 needs runs dense — v token
        # blocks 0-2 (block 3 must merely be EMITTED before av(kt3), i.e. by
        # the first 2-kt batch) and the pair-0 q/k blocks 0 and 4 (each in a
        # wqk-half split so they start as soon as the first half lands).
        # Everything else streams into the qt0 attention slots; a3/b3 lead
        # so v block 3 is emitted before av(kt3)'s flush.
        va_items, vb_items, qk_m0 = qkv_chunk_items(0, xt0, xt80, split_v=True)
        for f in va_items[:2]:
            f()
        for f in vb_items[:2]:
            f()
        for m in (0, 4):
            acc = qk_m0(m, irange=(0, 1))()
            qk_m0(m, irange=(2, 3), acc_in=acc)()
        rest0 = [va_items[2], vb_items[2], va_items[3], vb_items[3]] + \
                [qk_m0(m) for m in (1, 5, 2, 6, 3, 7)]

        def make_norm(pair, q0, av_E, av_O):
            def norm():
                # av -> SBUF so the PSUM banks free early; denominators
                # (E at partition 64, O at partition 0) broadcast over their
                # head's 64 partitions via K=1 PE outer products from legal
                # tile bases; reciprocal, scale, write both halves of aoT
                # directly (all per-partition: no partition-hop DMA).
                # av -> SBUF in bf16 (both the broadcast rhs and the final
                # scale read it; aoT is bf16 anyway)
                av8 = p2.tile([128, 2, TQ], BF16, tag="av8", bufs=2)
                nc.vector.tensor_copy(out=av8[:, 0, :], in_=av_E)
                nc.vector.tensor_copy(out=av8[:, 1, :], in_=av_O)
                # Denominator broadcasts as K=128 selector matmuls (cost is
                # N columns regardless of K/M) in plain 128-row mode: K=1
                # outer products are row-tiled and cannot write PSUM
                # partitions 64-127 (and corrupt a following 128-row matmul
                # mid-norm), so both broadcasts pick their denominator row
                # with a one-hot bf16 selector instead.
                bc = ps_acc.tile([128, TQ], F32, tag="acc")
                bc2 = ps_acc.tile([128, TQ], F32, tag="acc")
                _mm(nc, bc, sel64, av8[:, 0, :])
                _mm(nc, bc2, sel0, av8[:, 1, :])
                # custom-DVE ops (reciprocal) silently no-op on outputs
                # with base partition 64: run each reciprocal over the full
                # 128-partition range (both banks hold their denominator on
                # every partition) and slice at the multiply instead.
                bc_sbE = p2.tile([128, TQ], F32, tag="recbc", bufs=3)
                bc_sbO = p2.tile([128, TQ], F32, tag="recbc", bufs=3)
                nc.vector.reciprocal_approx_fast(out=bc_sbE[0:64],
                                                 in_=bc[0:64])
                nc.vector.reciprocal_approx_fast(out=bc_sbO, in_=bc2)
                nc.vector.tensor_mul(
                    out=aoTs[pair][0:64, q0 : q0 + TQ],
                    in0=av8[0:64, 0, :], in1=bc_sbE[0:64])
                nc.vector.tensor_mul(
                    out=aoTs[pair][64:128, q0 : q0 + TQ],
                    in0=av8[64:128, 1, :], in1=bc_sbO[64:128])
            return norm

        def make_proj(qt, ko_order=(0, 1, 2, 3), epilogue=False):
            q0 = qt * TQ
            def proj_m(m):
                def f():
                    acc = ps_acc.tile([128, TQ], F32, tag="acc")
                    for i, ko in enumerate(ko_order):
                        _mm(nc, acc, wo_sb[:, ko, m * 128 : (m + 1) * 128],
                            aoTs[ko][:, q0 : q0 + TQ], start=i == 0, stop=i == 3)
                    # fp16 output halves copy + store traffic; copies run on
                    # the DVE (ACT is exp-bound when proj runs), alternating
                    # with ACT in the post-last-exp epilogue
                    y_sb = p2.tile([128, TQ], FP16, tag="ysb", bufs=3)
                    if epilogue and m % 2 == 0:
                        nc.scalar.copy(out=y_sb, in_=acc)
                    else:
                        nc.vector.tensor_copy(out=y_sb, in_=acc)
                    eng = nc.gpsimd if epilogue and m % 2 else nc.sync
                    eng.dma_start(out=yT[qt, m], in_=y_sb)
                return f
            return [proj_m(m) for m in range(8)]

        # Pending PE work spread into the attention stream: next chunk's QKV
        # groups go anywhere (hard deadline: before the next q-tile), but
        # projection work is held back to the qt3 window where the scalar
        # engine's exp paces the pipeline and the PE needs filler.
        q_chunk = list(rest0)
        q_proj = []

        for qt in range(NQT):
            q0 = qt * TQ
            nkt = (q0 + TQ) // TK  # causal: only k-tiles with k0 <= q0+TQ-1
            if qt + 1 < NQT:
                q_chunk.extend(qkv_chunk_items(qt + 1, *load_chunk(qt + 1)))
            if qt == 0:  # out-proj weights needed from qt3: load after chunk-1
                nc.gpsimd.dma_start(out=wo_sb, in_=wo[:])
            # density credit: spread available items evenly over this qt's
            # k-tile slots (front-loading leaves ACT-paced dry stretches)
            slots = nkt * NPAIR
            pend_items = len(q_chunk) + (len(q_proj) if qt == NQT - 1 else 0)
            density = pend_items / slots
            credit = 0.40  # slight head start for the first slot
            pair_order = (1, 2, 3, 0) if qt == NQT - 1 else range(NPAIR)
            for pair in pair_order:
                qE = qkTs[qt][0:64, pair, :]
                qO = qkTs[qt][64:128, pair, :]
                av_E = ps_av.tile([128, TQ], F32, tag="avE")
                av_O = ps_av.tile([128, TQ], F32, tag="avO")

                def av_mms(e_sb, kt):
                    vc, vk = kt // (TQ // TK), kt % (TQ // TK)
                    c0 = max(0, kt * TK - q0)
                    stop = kt >= nkt - 4  # last write to each column range
                    nc.tensor.matmul(
                        av_E[:, c0:TQ],
                        r(v_augs[vc][:, vk, 2 * pair, :]),
                        r(e_sb[:, c0:TQ]), start=kt == 0, stop=stop,
                        skip_group_check=True)
                    nc.tensor.matmul(
                        av_O[:, c0:TQ],
                        r(v_augs[vc][:, vk, 2 * pair + 1, :]),
                        r(e_sb[:, TQ + c0 : 2 * TQ]), start=kt == 0, stop=stop,
                        skip_group_check=True)

                # 2-kt batches: both kts' score matmuls issue back-to-back in
                # the PE's 64-row mode, then all 128-row work (previous kts'
                # deferred av, chunk/proj items) follows — one mode-switch
                # pair per batch instead of per kt (each switch drains the
                # array). av lags one batch so the PE computes while ACT exps.
                pend = []
                for kt0 in range(0, nkt, 2):
                    kts = range(kt0, min(kt0 + 2, nkt))
                    cur = []
                    for kt in kts:
                        k0 = kt * TK
                        kc, kk = k0 // TQ, k0 % TQ
                        # causal column restriction: queries q0+c < k0 are
                        # fully masked for this k-tile: skip columns [0:c0)
                        c0 = max(0, k0 - q0)
                        s_ps = ps.tile([128, 2 * TQ], F32, tag="s")
                        for half, qh in ((0, qE), (1, qO)):
                            o0 = half * TQ
                            _mm(nc, s_ps[:, o0 + c0 : o0 + TQ],
                                qkTs[kc][64 * half : 64 * half + 64, 4 + pair,
                                         kk : kk + TK],
                                qh[:, c0:TQ])
                        cur.append((s_ps, kt, c0))
                    for s_ps, kt, c0 in cur:
                        e_sb = p2.tile([128, 2 * TQ], BF16, tag="e", bufs=4)
                        # e = exp(scores/sqrt(d_k)); no max-subtraction
                        # needed: scores/8 is O(1), exp cannot overflow.
                        nc.scalar.activation(
                            out=e_sb[:].rearrange("p (h q) -> p h q",
                                                  h=2)[:, :, c0:TQ],
                            in_=s_ps[:].rearrange("p (h q) -> p h q",
                                                  h=2)[:, :, c0:TQ],
                            func=mybir.ActivationFunctionType.Exp, scale=0.125)
                        if kt * TK >= q0:  # diagonal: zero masked entries
                            ev = e_sb[:].rearrange("p (h q) -> p h q", h=2)
                            nc.vector.tensor_mul(
                                out=ev[:, :, c0 : c0 + TK],
                                in0=ev[:, :, c0 : c0 + TK], in1=triK)
                        pend.append((e_sb, kt))
                    while len(pend) > 2:
                        av_mms(*pend.pop(0))
                    credit += density * len(kts)
                    while credit >= 1.0:
                        credit -= 1.0
                        # chunk work first (hard deadline), then projection
                        if q_chunk:
                            q_chunk.pop(0)()
                        elif q_proj and qt == NQT - 1:
                            q_proj.pop(0)()
                        else:
                            credit = 0.0
                for it in pend:
                    av_mms(*it)
                make_norm(pair, q0, av_E, av_O)()
                # pair-boundary fill: the next pair's first scores wait on
                # this pair's trailing exps (~1us); give the PE an item
                if credit >= 0.5:
                    if q_chunk:
                        credit -= 1.0
                        q_chunk.pop(0)()
                    elif q_proj and qt == NQT - 1:
                        credit -= 1.0
                        q_proj.pop(0)()
            if qt == NQT - 1:
                q_proj.extend(make_proj(qt, ko_order=(1, 2, 3, 0),
                                        epilogue=True))
            else:
                q_proj.extend(make_proj(qt))
        for f in q_chunk:
            f()
        for f in q_proj:
            f()
    nc.finalize()
    return nc


_CACHE = threading.local()


def _get_program():
    nc = getattr(_CACHE, "nc", None)
    if nc is None:
        nc = build_program()
        _CACHE.nc = nc
    return nc


def _make_in_maps(x, W_qkv, b_qkv, W_out, b_out):
    x = np.asarray(x, np.float32)
    W_qkv = np.asarray(W_qkv, np.float32)
    b_qkv = np.asarray(b_qkv, np.float32)
    W_out = np.asarray(W_out, np.float32)
    bf16 = ml_dtypes.bfloat16
    e4 = ml_dtypes.float8_e4m3

    def pack_x(xb):  # [T, C] -> [NQT, 2, 128, KH, TQ] (device access order)
        a = xb.T.reshape(KO, 128, NQT, TQ)          # (ko, p, ch, tt)
        a = a.transpose(2, 0, 1, 3)                 # (ch, ko, p, tt)
        a = a.reshape(NQT, 2, KH, 128, TQ)          # ko -> (h, kk)
        return a.transpose(0, 1, 3, 2, 4)           # (ch, h, p, kk, tt)

    def pack_w(w, nko):  # [nko*128, F] -> [128, nko, F]
        return w.reshape(nko, 128, -1).transpose(1, 0, 2)

    in_maps = []
    for c in range(NCORES):
        b, g = c // 2, c % 2
        sl = slice(512 * g, 512 * g + 512)  # this head group's q (and k,v) cols
        wqk_l = 32.0 * np.concatenate(
            [W_qkv[:, 0:1024][:, sl], W_qkv[:, 1024:2048][:, sl]], axis=1)
        bqk_l = np.concatenate([b_qkv[0:1024][sl], b_qkv[1024:2048][sl]])
        wv_l = W_qkv[:, 2048:3072][:, sl].reshape(2, KH, 128, CV)
        in_maps.append({
            "xT": np.ascontiguousarray(pack_x(x[b]).astype(bf16)),
            "x8T": np.ascontiguousarray(pack_x(x[b]).astype(e4)),
            "wqk": np.ascontiguousarray(pack_w(wqk_l, KO).astype(e4)),
            "bqk": np.ascontiguousarray(bqk_l.reshape(8, 128).T),
            "wv": np.ascontiguousarray(
                wv_l.transpose(0, 2, 1, 3).astype(bf16)),  # (h, p, kk, f)
            "bv": np.ascontiguousarray(b_qkv[2048:3072][sl]),
            "wo": np.ascontiguousarray(pack_w(W_out[sl, :], 4).astype(bf16)),
        })
    return in_maps


def _run(inputs, trace=False):
    nc = _get_program()
    in_maps = _make_in_maps(**inputs)
    res = run_bass_kernel_spmd(nc, in_maps, list(range(NCORES)), trace=trace)
    b_out = np.asarray(inputs["b_out"], np.float32)
    y = np.empty((B, T, C), np.float32)
    for b in range(B):
        # yT is [NQT, 8, 128, TQ] fp16: (qt, m, p, tt) -> [C, T] full
        yt = (res.results[2 * b]["yT"].astype(np.float32)
              + res.results[2 * b + 1]["yT"].astype(np.float32))
        y[b] = yt.transpose(0, 3, 1, 2).reshape(T, C) + b_out
    return y, res


def kernel(x, W_qkv, b_qkv, W_out, b_out):
    y, _ = _run(dict(x=x, W_qkv=W_qkv, b_qkv=b_qkv, W_out=W_out, b_out=b_out))
    return y
